# revision 20
# baseline (speedup 1.0000x reference)
"""GCN layer (nn_GCNLayer_72224170050097) as a Bass/Tile kernel on 8 TRN2 NeuronCores.

Math (reference):
    a_hat = adj + I
    d = rowsum(a_hat) ** -0.5
    out = (a_hat * d[:, None] * d[None, :]) @ x @ W.T + b

Sharding: 1D row-parallel over N=8192 (1024 rows per core).  Each core gets its
row-block of a_hat TRANSPOSED (contraction dim j on SBUF partitions, j = p*64+c
permutation baked into every staged operand - contraction is order invariant),
stored as an fp8-e4m3 hi+lo pair (same 16 MB as bf16, ~0.08% max residual).

The d-dependency is restructured so the AllGather hides completely:

    y = A @ (d * x) = A @ (mu * x) + A @ ((d - mu) * x),   mu = (N/2+1)^-1/2

  - U = A @ (mu*x) needs no degrees: it streams as fp8 DoubleRow matmuls
    (hi*hi + lo*hi + hi*lo; the lo*lo term is ~3e-4 relative, dropped) WHILE
    the adjT halves are still DMA-ing in.
  - The degree pass (ones^T @ A_hi, DoubleRow) completes as soon as the hi
    half has landed (~half the DMA phase), so the 4 KB degree AllGather and
    the rsqrt run under the lo-half DMA + U matmuls.
  - Only the small correction C = A_hi @ ((d-mu)*x) (one DoubleRow pass,
    |d-mu| ~ 0.4% of mu) remains after the collective.
  - Epilogue: y = (U*KU + C*KC) * d_row, then W matmul (bf16), + bias.

Scale bookkeeping (fp8 e4m3 underflows below ~2e-3, so small terms are staged
pre-scaled):  q = SX*x with SX = 64*mu ~ 1.0 (host);  xs2 = SD*(d-mu) * q
-> on-device combine  y = KU*U_acc + KC*C_acc,  KU = mu/SX = 1/64,
KC = 1/(SX*SD).
"""

import sys

if "/opt/trn_rl_repo" not in sys.path:
    sys.path.insert(0, "/opt/trn_rl_repo")

import numpy as np
import ml_dtypes

import concourse.bass as bass
import concourse.mybir as mybir
import concourse.tile as tile
from concourse import bacc
from concourse.bass_utils import run_bass_kernel_spmd

N = 8192
D = 128
NCORES = 8
NB = N // NCORES  # 1024 rows per core
P = 128
C = N // P  # 64 chunks of the contraction dim
H = NB // 512  # 2 free-dim halves of 512
G = 8  # chunks per adjT DMA (1 MiB fp8 transfers, 8KB contiguous runs)

MU = float((N / 2 + 1) ** -0.5)
SX = 64.0 * MU  # host scale on x (~1.0)
SD = 4096.0  # device scale on (d - mu)
KU = MU / SX  # = 1/64
KC = 1.0 / (SX * SD)

dt = mybir.dt
BF16 = ml_dtypes.bfloat16
F8 = ml_dtypes.float8_e4m3

_CACHE = {}


def _emit_body(nc, pools, aps, rep):
    atpool, sb, ps, dram = pools
    ahi3, alo3, xhi2, xlo2, wt, bias, outT = aps
    r = f"_{rep}"
    DR = mybir.MatmulPerfMode.DoubleRow

    # DoubleRow LDW needs all 128 PE columns active (col_grp=0xf) and a
    # 16B-aligned k-pair step, so the degree weights are a full [128,2,128]
    # ones block; the degree lands replicated across PSUM partitions.
    ones2 = sb.tile([P, 2, P], dt.float8e4, tag="ones2", name="ones2" + r)
    nc.vector.memset(ones2[:], 1.0)

    # small DMAs on the ACT queue (SP streams adjT continuously)
    xhi = sb.tile([P, C, D], dt.float8e4, tag="xhi", name="xhi" + r)
    nc.scalar.dma_start(xhi[:], xhi2)
    xlo = sb.tile([P, C, D], dt.float8e4, tag="xlo", name="xlo" + r)
    nc.scalar.dma_start(xlo[:], xlo2)
    wts = sb.tile([D, D], dt.bfloat16, tag="wts", name="wts" + r)
    nc.scalar.dma_start(wts[:], wt)
    bs = sb.tile([D, 1], dt.float32, tag="bs", name="bs" + r)
    nc.scalar.dma_start(bs[:], bias)

    pdeg = [
        ps.tile([P, 512], dt.float32, tag=f"pdeg{h}", name=f"pdeg{h}{r}")
        for h in range(H)
    ]
    py = [
        ps.tile([P, 512], dt.float32, tag=f"py{h}", name=f"py{h}{r}")
        for h in range(H)
    ]
    pyc = [
        ps.tile([P, 512], dt.float32, tag=f"pyc{h}", name=f"pyc{h}{r}")
        for h in range(H)
    ]

    # ---- hi half: DMA + degree pass + U (hi*hi, lo*hi) ----
    NG = C // G  # 8 tile groups per half
    ahi_tiles = []
    first_at_inst = None
    for g in range(NG):
        at = atpool.tile([P, G, NB], dt.float8e4, tag="ahi", name=f"ahi{g}{r}")
        dma_inst = nc.sync.dma_start(at[:], ahi3[:, g * G : (g + 1) * G, :])
        if first_at_inst is None:
            first_at_inst = dma_inst
        ahi_tiles.append(at)
        for qp in range(G // 2):
            cp = g * (G // 2) + qp  # chunk-pair index, 0..31
            rhs = at[:, 2 * qp : 2 * qp + 2, :]
            for h in range(H):
                hs = slice(h * 512, (h + 1) * 512)
                # degrees (from the hi half only; ~1e-4 relative is plenty)
                nc.tensor.matmul(
                    pdeg[h][:],
                    lhsT=ones2[:],
                    rhs=rhs[:, :, hs],
                    start=(cp == 0),
                    stop=(cp == C // 2 - 1),
                    perf_mode=DR,
                )
                # U += A_hi @ q_hi
                nc.tensor.matmul(
                    py[h][:],
                    lhsT=xhi[:, 2 * cp : 2 * cp + 2, :],
                    rhs=rhs[:, :, hs],
                    start=(cp == 0),
                    stop=False,
                    perf_mode=DR,
                )
                # U += A_hi @ q_lo
                nc.tensor.matmul(
                    py[h][:],
                    lhsT=xlo[:, 2 * cp : 2 * cp + 2, :],
                    rhs=rhs[:, :, hs],
                    start=False,
                    stop=False,
                    perf_mode=DR,
                )

    # raw degrees -> SBUF (DVE) -> DRAM (ACT queue; SP is busy with the lo
    # half) -> AllGather.  All of this hides under the lo-half DMA.
    degloc = sb.tile([1, NB], dt.float32, tag="degloc", name="degloc" + r)
    for h in range(H):
        nc.vector.tensor_copy(degloc[:, h * 512 : (h + 1) * 512], pdeg[h][0:1, :])
    degloc_d = dram.tile([1, NB], dt.float32, tag="degloc_d", name="degloc_d" + r)
    # split the single-partition 4KB DMA across two queues (it runs at ~1
    # partition-port of bandwidth, so halving it halves the latency)
    nc.scalar.dma_start(degloc_d[:, :512], degloc[:, :512])
    nc.gpsimd.dma_start(degloc_d[:, 512:], degloc[:, 512:])
    degfull_d = dram.tile(
        [NCORES, NB], dt.float32, tag="degfull_d", name="degfull_d" + r
    )
    nc.gpsimd.collective_compute(
        "AllGather",
        mybir.AluOpType.bypass,
        replica_groups=[list(range(NCORES))],
        ins=[degloc_d[:].opt()],
        outs=[degfull_d[:].opt()],
    )

    # ---- lo half: DMA + U (hi-x * lo-A) ----
    for g in range(NG):
        at = atpool.tile([P, G, NB], dt.float8e4, tag="alo", name=f"alo{g}{r}")
        nc.sync.dma_start(at[:], alo3[:, g * G : (g + 1) * G, :])
        for qp in range(G // 2):
            cp = g * (G // 2) + qp
            for h in range(H):
                nc.tensor.matmul(
                    py[h][:],
                    lhsT=xhi[:, 2 * cp : 2 * cp + 2, :],
                    rhs=at[:, 2 * qp : 2 * qp + 2, h * 512 : (h + 1) * 512],
                    start=False,
                    stop=(cp == C // 2 - 1),
                    perf_mode=DR,
                )

    # this core's KU*d (output row scale) on 128 lanes via a [128, 8] DRAM
    # round-trip (degloc_d is already in DRAM); all off the critical path.
    # Rsqrt on ACT is banned for accuracy -> sqrt + recip.
    dg2 = sb.tile([P, 8], dt.float32, tag="dg2", name="dg2" + r)
    nc.scalar.dma_start(dg2[:], degloc_d[:].rearrange("a (p t) -> (a p) t", t=8))
    ds2 = sb.tile([P, 8], dt.float32, tag="ds2", name="ds2" + r)
    nc.scalar.activation(ds2[:], dg2[:], mybir.ActivationFunctionType.Sqrt)
    dr2 = sb.tile([P, 8], dt.float32, tag="dr2", name="dr2" + r)
    nc.vector.reciprocal(dr2[:], ds2[:])
    dk2 = sb.tile([P, 8], dt.float32, tag="dk2", name="dk2" + r)
    nc.vector.tensor_scalar_mul(dk2[:], dr2[:], KU)
    dloc_d = dram.tile([1, NB], dt.float32, tag="dloc_d", name="dloc_d" + r)
    nc.scalar.dma_start(
        dloc_d[:].rearrange("a (p t) -> (a p) t", t=8), dk2[:]
    )
    drep = sb.tile([P, NB], dt.float32, tag="drep", name="drep" + r)
    nc.gpsimd.dma_start(drep[:], dloc_d[:].to_broadcast([P, NB]))

    # post-collective: wide rsqrt, then delta2 = SD*(d - mu)
    Dg = sb.tile([P, C], dt.float32, tag="Dg", name="Dg" + r)
    nc.scalar.dma_start(Dg[:], degfull_d[:].rearrange("k (pp c) -> (k pp) c", c=C))
    Dsq = sb.tile([P, C], dt.float32, tag="Dsq", name="Dsq" + r)
    nc.scalar.activation(Dsq[:], Dg[:], mybir.ActivationFunctionType.Sqrt)
    Dsb = sb.tile([P, C], dt.float32, tag="Dsb", name="Dsb" + r)
    nc.vector.reciprocal(Dsb[:], Dsq[:])
    Dd = sb.tile([P, C], dt.bfloat16, tag="Dd", name="Dd" + r)
    nc.vector.tensor_scalar(
        Dd[:], Dsb[:], SD, -SD * MU, mybir.AluOpType.mult, mybir.AluOpType.add
    )

    # xs2 = delta2 * x_hi (fp8; the delta2*x_lo term is ~2e-4 relative and is
    # dropped), in slabs so the C pass can start early
    xs2 = sb.tile([P, C, D], dt.float8e4, tag="xs2", name="xs2" + r)
    SL = 16
    for s in range(C // SL):
        sl = slice(s * SL, (s + 1) * SL)
        nc.vector.tensor_tensor(
            xs2[:, sl, :],
            xhi[:, sl, :],
            Dd[:, sl, None].to_broadcast([P, SL, D]),
            mybir.AluOpType.mult,
        )

    # ---- correction pass + epilogue, h-outer so half-0's epilogue overlaps
    # half-1's correction matmuls ----
    yt = sb.tile([P, NB], dt.bfloat16, tag="yt", name="yt" + r)
    osb = sb.tile([D, NB], dt.float32, tag="osb", name="osb" + r)
    out_inst = None
    for h in range(H):
        hs = slice(h * 512, (h + 1) * 512)
        for cp in range(C // 2):
            g, qp = cp // (G // 2), cp % (G // 2)
            nc.tensor.matmul(
                pyc[h][:],
                lhsT=xs2[:, 2 * cp : 2 * cp + 2, :],
                rhs=ahi_tiles[g][:, 2 * qp : 2 * qp + 2, hs],
                start=(cp == 0),
                stop=(cp == C // 2 - 1),
                perf_mode=DR,
            )
        # yt = (U + (KC/KU)*C) * (KU*d_row)   (KU folded into drep)
        t1 = sb.tile([P, 512], dt.float32, tag="t1", name=f"t1_{h}{r}")
        nc.vector.tensor_scalar_mul(t1[:], pyc[h][:], KC / KU)
        t2 = sb.tile([P, 512], dt.float32, tag="t2", name=f"t2_{h}{r}")
        nc.vector.tensor_tensor(t2[:], t1[:], py[h][:], mybir.AluOpType.add)
        nc.vector.tensor_tensor(yt[:, hs], t2[:], drep[:, hs], mybir.AluOpType.mult)
        pz = ps.tile([P, 512], dt.float32, tag=f"pz{h}", name=f"pz{h}{r}")
        nc.tensor.matmul(
            pz[:], lhsT=wts[:], rhs=yt[:, hs], start=True, stop=True
        )
        nc.vector.tensor_scalar_add(osb[:, hs], pz[:], bs[:])
        out_inst = nc.sync.dma_start(outT[:, hs], osb[:, hs])
    return first_at_inst, out_inst


def build_nc(reps=None):
    """reps=None -> single body (production).  reps=R -> body statically
    unrolled R times, serialized, for slope timing."""
    nc = bacc.Bacc(
        "TRN2",
        target_bir_lowering=False,
        debug=False,
        num_devices=NCORES,
    )
    ahi = nc.dram_tensor("ahi", [N, NB], dt.float8e4, kind="ExternalInput").ap()
    alo = nc.dram_tensor("alo", [N, NB], dt.float8e4, kind="ExternalInput").ap()
    xhi = nc.dram_tensor("xhi", [N, D], dt.float8e4, kind="ExternalInput").ap()
    xlo = nc.dram_tensor("xlo", [N, D], dt.float8e4, kind="ExternalInput").ap()
    wt = nc.dram_tensor("wt", [D, D], dt.bfloat16, kind="ExternalInput").ap()
    bias = nc.dram_tensor("bias", [D, 1], dt.float32, kind="ExternalInput").ap()
    outT = nc.dram_tensor("outT", [D, NB], dt.float32, kind="ExternalOutput").ap()

    with tile.TileContext(nc) as tc:
        with (
            tc.tile_pool(name="at", bufs=C // G) as atpool,
            tc.tile_pool(name="sb", bufs=1) as sb,
            tc.tile_pool(name="ps", bufs=1, space="PSUM") as ps,
            tc.tile_pool(name="dram", bufs=1, space="DRAM") as dram,
        ):
            aps = (
                ahi.rearrange("(p c) i -> p c i", c=C),
                alo.rearrange("(p c) i -> p c i", c=C),
                xhi.rearrange("(p c) f -> p c f", c=C),
                xlo.rearrange("(p c) f -> p c f", c=C),
                wt,
                bias,
                outT,
            )
            pools = (atpool, sb, ps, dram)
            prev_out = None
            for rep in range(reps or 1):
                first, out = _emit_body(nc, pools, aps, rep)
                if prev_out is not None:
                    bass._add_dep_helper(
                        first.ins, prev_out.ins, sync=True,
                        reason="timing: serialize reps",
                    )
                prev_out = out

    nc.compile()
    return nc


def get_nc():
    if "nc" not in _CACHE:
        _CACHE["nc"] = build_nc()
    return _CACHE["nc"]


def make_in_maps(x, adj, W, b):
    x = np.asarray(x, dtype=np.float32)
    adj = np.asarray(adj, dtype=np.float32)
    W = np.asarray(W, dtype=np.float32)
    b = np.asarray(b, dtype=np.float32)

    xq = (SX * x).astype(np.float32)
    xhi = xq.astype(F8)
    xlo = (xq - xhi.astype(np.float32)).astype(F8)
    wt16 = np.ascontiguousarray(W.T).astype(BF16)
    bias32 = np.ascontiguousarray(b.reshape(D, 1))

    in_maps = []
    idx = np.arange(NB)
    for k in range(NCORES):
        blk = adj[k * NB : (k + 1) * NB, :]  # [NB, N]
        a32 = np.ascontiguousarray(blk.T)  # [N, NB]
        a32[k * NB + idx, idx] += 1.0  # bake the +I diagonal
        ahi = a32.astype(F8)
        alo = (a32 - ahi.astype(np.float32)).astype(F8)
        in_maps.append(
            {
                "ahi": ahi,
                "alo": alo,
                "xhi": xhi,
                "xlo": xlo,
                "wt": wt16,
                "bias": bias32,
            }
        )
    return in_maps


def kernel(**inputs) -> np.ndarray:
    nc = get_nc()
    in_maps = make_in_maps(inputs["x"], inputs["adj"], inputs["W"], inputs["b"])
    res = run_bass_kernel_spmd(nc, in_maps, list(range(NCORES)))
    out = np.empty((N, D), dtype=np.float32)
    for k in range(NCORES):
        out[k * NB : (k + 1) * NB, :] = res.results[k]["outT"].T
    return out
